# revision 76
# baseline (speedup 1.0000x reference)
"""Trainium2 Bass kernel for the CDGR gnn_message_passing module.

Mathematically exact reformulation of the reference:
  - softmax rows of A sum to 1  =>  L = I - A, the d-scaling vanishes
  - s2l logits are additively separable in (pixel, node) => the softmax
    over pixels is identical for every node column => app collapses to a
    rank-1 outer product relu(G) (x) softmax(w_in . x)
  - the exp-overflow shift folds into the S^T matmul lhsT:
    S - ub = sum_k (Dg@R - rowmax(Dg@R))[k,q] * x_phi^T[k,p]
  - the semantic branch (word attention + 2-layer GCN) is batch
    independent => computed once per core
  - the two chained 1x1 convs fuse: Wlg = final_w[:, :C] @ gw_w

Performance structure (135.4us -> 70.4us on the timeline cost model):
  - all weights packed host-side into one [128, NCOL] DRAM tensor;
    the hot (phi) section + attention-critical head load early via SP,
    the rest trickles through the Pool/SWDGE queue in ~1.3us chunks so
    the single DMA-engine device (exclusive FIFO by request order)
    never convoys the latency-critical R-reshape roundtrip
  - the torch-.view reshape of relu(phi) goes DRAM-roundtrip on the
    Pool queue, sequenced ahead of the weight trickle
  - one activation-table load: first act op is a dummy Exp (pins
    exp_and_friends); sigmoid is computed as 1/(1+exp(-z)), rsqrt by
    Newton iteration on DVE, R-relu+bias as one DVE tensor_scalar
  - the final +x residual enters the output matmul as an extra
    identity-matrix matmul term (PSUM accumulate); outputs are
    computed, relu'd and stored in independent 512-pixel halves that
    overlap the second batch's E@xv accumulation
  - two batches' pipelines and the semantic chain interleave at tile
    granularity (zip1/zip2); tc.tile_wait_until pseudo-times pin the
    TileScheduler's per-engine queue order where its internal readiness
    model would otherwise head-block an engine
  - 20 dependency-free warmup transposes hold the PE at full p-state
    before the first real matmul
"""

import os
from contextlib import ExitStack

import numpy as np

import concourse.bass as bass
import concourse.bacc as bacc
import concourse.mybir as mybir
import concourse.tile as tile
from concourse import masks
from concourse.bass_utils import run_bass_kernel_spmd

FP = mybir.dt.float32
FR = mybir.dt.float32r
AF = mybir.ActivationFunctionType
ALU = mybir.AluOpType

NCORES = 8
BPC = 2          # batches per core
C, HW = 256, 1024
MPHI, NN, DE = 16, 20, 300
KE = DE + 1      # 301 = DEMB + fused-bias row

LAST_EXEC_NS = None
LAST_RESULT = None


# --------------------------------------------------------------------------
# weight pack layout (shared between host prep and device build)
# --------------------------------------------------------------------------

def _ksl(total, step=128):
    return [(o, min(step, total - o)) for o in range(0, total, step)]


def _pack_layout():
    """Column-pack every weight into one [128, NCOL] array.
    Returns (layout, section_bounds): layout[key] = (r0, c0, rows, cols);
    sections = [(c_start, c_end)] for the 3 load DMAs."""
    L = {}
    col = [0]

    def place(key, rows, cols, r0=0, at=None):
        c0 = col[0] if at is None else at
        L[key] = (r0, c0, rows, cols)
        if at is None:
            col[0] += cols

    bounds = []
    mark = [0]

    def close_section():
        bounds.append((mark[0], col[0]))
        mark[0] = col[0]

    # --- section 1: hot (needed by every batch immediately) ---
    place("phiwT0", 128, MPHI)
    place("phiwT1", 128, MPHI)
    place("globwT0", 128, MPHI)
    place("globwT1", 128, MPHI)
    place("win0", 128, 1)
    place("win1", 128, 1)
    place("phib", MPHI, 1)
    close_section()

    # --- section 2: attention / embedding, ordered by first use so the
    # chunked trickle load unblocks the semantic chain ASAP ---
    place("embTe0", 128, NN)
    place("embTe1", 128, NN)
    place("embTe2", 45, NN)
    for w in ("wq", "wk"):
        place(f"{w}0", 128, DE)
        place(f"{w}1", 128, DE)
        place(f"{w}2", 44, DE)
    for b in ("bq", "bk"):
        place(f"{b}0", 128, 1)
        place(f"{b}1", 128, 1)
        place(f"{b}2", 44, 1)
    place("wve0", 128, DE)
    place("wve1", 128, DE)
    place("wve2", 45, DE)
    place("adj", NN, NN)
    place("wo0", 128, DE)
    place("wo1", 128, DE)
    # matmul operands need base partition 0; DVE-only emb/bo_row stack
    sA = col[0]
    place("wo2", 44, DE, r0=0, at=sA)
    place("emb", NN, DE, r0=64, at=sA)
    place("bo_row", 1, DE, r0=96, at=sA)
    col[0] += DE
    close_section()

    # --- section 3: gcn + output weights ---
    place("gc10", 128, C)
    place("gc11", 128, C)
    place("gc12", 44, C)
    place("gc20", 128, C)
    place("gc21", 128, C)
    place("gww0", 128, C)
    place("gww1", 128, C)
    place("fwT0", 128, C)
    place("fwT1", 128, C)
    place("fwT2", 128, C)
    place("fwT3", 128, C)
    close_section()

    return L, bounds, col[0]


_LAYOUT, _SECTIONS, _NCOL = _pack_layout()


def _prep_shared(inputs):
    f = lambda k: np.ascontiguousarray(inputs[k], dtype=np.float32)
    pack = np.zeros((128, _NCOL), np.float32)

    def put(key, arr):
        r0, c0, rows, cols = _LAYOUT[key]
        assert arr.shape == (rows, cols), (key, arr.shape, (rows, cols))
        pack[r0:r0 + rows, c0:c0 + cols] = arr

    phiwT = f("phi_w").T          # [C, 16]
    globwT = f("glob_w").T
    win = f("s2l_w")[:C].reshape(C, 1)
    put("phiwT0", phiwT[:128]); put("phiwT1", phiwT[128:])
    put("globwT0", globwT[:128]); put("globwT1", globwT[128:])
    put("win0", win[:128]); put("win1", win[128:])
    put("phib", f("phi_b").reshape(MPHI, 1))

    embTe = np.ascontiguousarray(
        np.vstack([f("emb").T, np.ones((1, NN), np.float32)]))  # [301, 20]
    wve = np.ascontiguousarray(np.vstack([f("wv"), f("bv")[None, :]]))  # [301, 300]
    for name, arr in (("wq", f("wq")), ("wk", f("wk")), ("wve", wve),
                      ("wo", f("wo"))):
        for i, (o, s) in enumerate(_ksl(arr.shape[0])):
            put(f"{name}{i}", arr[o:o + s])
    put("bo_row", f("bo").reshape(1, DE))
    put("emb", f("emb"))
    for i, (o, s) in enumerate(_ksl(KE)):
        put(f"embTe{i}", embTe[o:o + s])
    put("adj", f("adj"))
    for name, arr in (("bq", f("bq").reshape(DE, 1)), ("bk", f("bk").reshape(DE, 1))):
        for i, (o, s) in enumerate(_ksl(DE)):
            put(f"{name}{i}", arr[o:o + s])

    for name, arr in (("gc1", f("gc1_w")), ("gc2", f("gc2_w")), ("gww", f("gw_w")),
                      ("fwT", np.ascontiguousarray(f("final_w").T))):
        for i, (o, s) in enumerate(_ksl(arr.shape[0])):
            put(f"{name}{i}", arr[o:o + s])

    return {"wpack": pack}


# --------------------------------------------------------------------------
# device program
# --------------------------------------------------------------------------

def _fr(ap):
    return ap.bitcast(FR)


def _build_nc(reps=1):
    nc = bacc.Bacc()

    def par(name, shape, out=False):
        return nc.declare_dram_parameter(name, list(shape), FP, isOutput=out)

    x_p = par("x", [BPC, C * HW])
    out_p = par("out", [BPC, C * HW], out=True)
    wpack_p = par("wpack", [128, _NCOL])
    rscr = nc.dram_tensor("rscratch", [reps * BPC, MPHI * HW], FP)

    with tile.TileContext(nc) as tc:
        with nc.allow_low_precision(reason="float32r matmul feed tags"), \
             ExitStack() as ctx:
            _body(ctx, tc, nc, x_p, out_p, wpack_p, rscr, reps=reps)
    nc.finalize()
    return nc


def _body(ctx, tc, nc, x_p, out_p, wpack_p, rscr, reps=1):
    mm = lambda out, lhsT, rhs, start, stop: nc.tensor.matmul(
        out, lhsT.bitcast(FR), rhs.bitcast(FR), start=start, stop=stop)

    cw = ctx.enter_context(tc.tile_pool(name="cw", bufs=1))       # weights/persistent
    sm = ctx.enter_context(tc.tile_pool(name="sm", bufs=2))       # small working
    sprp = ctx.enter_context(tc.tile_pool(name="sprp", bufs=4))   # spiral row tiles
    etp = ctx.enter_context(tc.tile_pool(name="etp", bufs=16))    # E^T tiles, per-batch slots
    spp = ctx.enter_context(tc.tile_pool(name="spp", bufs=4))     # spiral^T tiles
    obp = ctx.enter_context(tc.tile_pool(name="obp", bufs=6))     # output staging
    ps_big = ctx.enter_context(tc.tile_pool(name="ps_big", bufs=2, space="PSUM"))
    ps_exv = ctx.enter_context(tc.tile_pool(name="ps_exv", bufs=2, space="PSUM"))
    ps_sml = ctx.enter_context(tc.tile_pool(name="ps_sml", bufs=2, space="PSUM"))

    # ---- persistent weight tiles: one per section, sliced via layout ----
    wsec = []
    for si, (c0, c1) in enumerate(_SECTIONS):
        t = cw.tile([128, c1 - c0], FP, tag=f"wsec{si}", name=f"wsec{si}")
        wsec.append((t, c0, c1))

    def W(key):
        r0, c0, rows, cols = _LAYOUT[key]
        for t, s0, s1 in wsec:
            if s0 <= c0 < s1:
                return t[r0:r0 + rows, c0 - s0:c0 - s0 + cols]
        raise KeyError(key)

    # ---- consts (no DMA: memset / iota) ----
    identr = cw.tile([128, 128], FP, tag="identr")
    masks.make_identity(nc, identr[:])
    ident = cw.tile([128, 128], FP, tag="ident")
    nc.vector.tensor_copy(_fr(ident[:]), identr[:])
    rawr20 = cw.tile([1, NN], FP, tag="rawr20")
    nc.vector.memset(rawr20[:], 1.0)
    one_row20 = cw.tile([1, NN], FP, tag="one_row20")
    nc.vector.tensor_copy(_fr(one_row20[:]), rawr20[:])
    rawc20 = cw.tile([NN, 8], FP, tag="rawc20")
    nc.vector.memset(rawc20[:], 1.0)
    ones20 = cw.tile([NN, 8], FP, tag="ones20")
    nc.vector.tensor_copy(_fr(ones20[:]), rawc20[:])
    inv20 = cw.tile([NN, 8], FP, tag="inv20")
    nc.vector.tensor_scalar_mul(_fr(inv20[:]), rawc20[:], 1.0 / NN)
    rawcol8 = cw.tile([128, 2], FP, tag="rawcol8")
    nc.vector.memset(rawcol8[:], 0.0)
    nc.vector.memset(rawcol8[:, 0:1], 1.0)
    junk1 = cw.tile([1, 1], FP, tag="junk1")
    # dummy exp: pins the single activation table (exp_and_friends) at t=0
    nc.scalar.activation(junk1[:], ident[0:1, 0:1], AF.Exp)
    # PE p-state warmup: dependency-free transposes ramp the tensor
    # engine to full clock before the first real matmul arrives
    for wu in range(int(os.environ.get("KWU", "20"))):
        ps_wu = ps_sml.tile([128, 128], FP, tag="ps_sml", name=f"wu{wu}")
        nc.tensor.transpose(_fr(ps_wu[:]), _fr(ident[:, :]), _fr(ident[:, :]))

    # ---- per-batch persistent tiles ----
    xmat = []    # [128, 2048] (c within chunk, (cc q))
    xvt = []     # [128, 8*257] (q within tile, (k: c+ones))
    Rt = []      # [16, 1024]
    xpa = []     # [128, 128]  ((k j) reshaped x_phi rows)
    Me = []      # [17, 1024]
    xpT = []     # [17, 1024]
    ET = []      # 8 x [128, 1024]
    spT = []     # 2 x [128, 1024]
    for b in range(BPC):
        xmat.append(cw.tile([128, 2 * HW], FP, tag=f"xmat{b}", name=f"xmat{b}"))
        xvt.append(cw.tile([128, 8 * (C + 2)], FP, tag=f"xvt{b}", name=f"xvt{b}"))
        Rt.append(cw.tile([MPHI, HW], FP, tag=f"Rt{b}", name=f"Rt{b}"))
        xpa.append(cw.tile([128, 8 * MPHI], FP, tag=f"xpa{b}", name=f"xpa{b}"))
        Me.append(cw.tile([MPHI, HW], FP, tag=f"Me{b}", name=f"Me{b}"))
        xpT.append(cw.tile([MPHI, HW], FP, tag=f"xpT{b}", name=f"xpT{b}"))
        ET.append([etp.tile([128, HW], FP, tag="et", name=f"ET{b}_{k}")
                   for k in range(8)])
        spT.append([spp.tile([128, HW], FP, tag="spTc", name=f"spT{b}_{i}")
                    for i in range(2)])
        # const tail columns of xv: col 256 = 1 (row-sum D), 257..263 = 0
        xv3 = xvt[b][:].rearrange("p (k c) -> p k c", c=C + 2)
        for k in range(8):
            nc.vector.tensor_copy(_fr(xv3[:, k, C:C + 2]), rawcol8[:])

    # ---- DMA loads. The DMA-engine device is an exclusive FIFO: big
    # transfers issued early would convoy urgent small ones, so only the
    # hot section + x go on SP up front; the two big weight sections
    # trickle through the Pool/SWDGE queue in ~1.3us chunks whose device
    # requests arrive late enough for latency-critical DMAs to jump ahead.
    x_cq = x_p[:].rearrange("b (c q) -> b c q", c=C)       # [b, 256, 1024]
    x_pc = x_p[:].rearrange("b (p c) -> b p c", c=C)       # [b, 1024, 256] (flat regroup)
    out_cq = out_p[:].rearrange("b (c q) -> b c q", c=C)
    r_jq = rscr[:].rearrange("b (j q) -> b j q", j=MPHI)
    r_pj = rscr[:].rearrange("b (p j) -> b p j", j=MPHI)

    def load_xm(b):
        # two DMAs (one per channel half) so phi's first matmuls can
        # start as soon as the first half lands
        xm3 = xmat[b][:].rearrange("c (cc q) -> c cc q", q=HW)
        xs = x_cq[b].rearrange("(cc c) q -> c cc q", c=128)
        for cc in range(2):
            nc.sync.dma_start(_fr(xm3[:, cc, :]), _fr(xs[:, cc, :]))

    def trickle(si, step=640, start=0):
        t, c0, c1 = wsec[si]
        w = c1 - c0
        for a in range(start, w, step):
            b_ = min(a + step, w)
            nc.gpsimd.dma_start(_fr(t[:, a:b_]), _fr(wpack_p[:][:, c0 + a:c0 + b_]))

    W1HEAD = 3 * NN + 2 * (2 * DE + 44) + 3  # embTe + wq chunks + bq cols

    def load_xv(b):
        xv3 = xvt[b][:].rearrange("p (k c) -> p k c", c=C + 2)
        nc.gpsimd.dma_start(_fr(xv3[:, :, 0:C]),
                            _fr(x_pc[b].rearrange("(k p) c -> p k c", p=128)))

    nc.sync.dma_start(_fr(wsec[0][0][:]),
                      _fr(wpack_p[:][:, wsec[0][1]:wsec[0][2]]))
    load_xm(0)
    load_xm(1)
    nc.sync.dma_start(_fr(wsec[1][0][:, 0:W1HEAD]),
                      _fr(wpack_p[:][:, wsec[1][1]:wsec[1][1] + W1HEAD]))

    # ================= stage emitters =================

    def front(b, rs):
        """phi -> R -> DRAM roundtrip reshape into xpa -> xmean.
        Both roundtrip DMAs ride the Pool/SWDGE queue AHEAD of the weight
        trickle so their DMA-device requests are never convoyed."""
        ps_phi = ps_big.tile([MPHI, HW], FP, tag="ps_big", name=f"psphi{b}")
        phiwT = (W("phiwT0"), W("phiwT1"))
        for ki in range(2):
            for nh in range(2):
                mm(ps_phi[:, 512 * nh:512 * (nh + 1)],
                   phiwT[ki], xmat[b][:, ki * HW + 512 * nh:ki * HW + 512 * (nh + 1)],
                   start=(ki == 0), stop=(ki == 1))
        nc.vector.tensor_scalar(_fr(Rt[b][:]), ps_phi[:], W("phib")[:, 0:1], 0.0,
                                op0=ALU.add, op1=ALU.max)
        nc.gpsimd.dma_start(r_jq[rs], Rt[b][:])
        nc.gpsimd.dma_start(
            _fr(xpa[b][:].rearrange("p (k j) -> p k j", j=MPHI)),
            _fr(r_pj[rs].rearrange("(k p) j -> p k j", p=128)))
        xmean = sm.tile([128, 16], FP, tag=f"xmean{b}", name=f"xmean{b}", bufs=1)
        nc.vector.memset(xmean[:], 0.0)
        for ki in range(2):
            nc.vector.tensor_reduce(_fr(xmean[:, 8 * ki:8 * ki + 1]),
                                    xmat[b][:, ki * HW:(ki + 1) * HW],
                                    axis=mybir.AxisListType.X, op=ALU.add)
        return xmean

    def dgme(b, xmean):
        """Dg -> Me = Dg @ R - Mmax (exp-overflow shift folded into lhsT)"""
        ps_g = ps_sml.tile([MPHI, 8], FP, tag="ps_sml", name=f"psg{b}")
        globwT = (W("globwT0"), W("globwT1"))
        for ki in range(2):
            mm(ps_g[:], globwT[ki], xmean[:, 8 * ki:8 * ki + 8],
               start=(ki == 0), stop=(ki == 1))
        eng = sm.tile([MPHI, 1], FP, tag=f"eng{b}", name=f"eng{b}", bufs=1)
        nc.scalar.activation(eng[:], ps_g[:, 0:1], AF.Exp, scale=float(-1.0 / HW))
        ep1 = sm.tile([MPHI, 1], FP, tag=f"ep1{b}", name=f"ep1{b}", bufs=1)
        nc.vector.tensor_scalar_add(ep1[:], eng[:], 1.0)
        sgm = sm.tile([MPHI, 1], FP, tag=f"sgm{b}", name=f"sgm{b}", bufs=1)
        nc.vector.reciprocal(sgm[:], ep1[:])
        sm05 = sm.tile([MPHI, 1], FP, tag=f"sm05{b}", name=f"sm05{b}", bufs=1)
        nc.vector.tensor_scalar_add(sm05[:], sgm[:], -0.5)
        Dg = sm.tile([MPHI, MPHI], FP, tag=f"Dg{b}", name=f"Dg{b}", bufs=1)
        nc.vector.tensor_scalar(_fr(Dg[:]), ident[0:MPHI, 0:MPHI], sm05[:, 0:1], 0.5,
                                op0=ALU.mult, op1=ALU.add)
        ps_m = ps_big.tile([MPHI, HW], FP, tag="ps_big", name=f"psm{b}")
        for nh in range(2):
            mm(ps_m[:, 512 * nh:512 * (nh + 1)], Dg[:, :],
               Rt[b][:, 512 * nh:512 * (nh + 1)], start=True, stop=True)
        Mmax = sm.tile([MPHI, 1], FP, tag=f"Mmax{b}", name=f"Mmax{b}", bufs=1)
        nc.vector.tensor_reduce(Mmax[:], ps_m[:], axis=mybir.AxisListType.X,
                                op=ALU.max)
        nc.vector.tensor_scalar_sub(_fr(Me[b][:]), ps_m[:], Mmax[:, 0:1])

    def xpt(b):
        """xpT by PE transpose of xpa 16-col blocks (chunked copies)"""
        for ch in range(2):
            ps_t = ps_sml.tile([MPHI, 512], FP, tag="ps_sml", name=f"psxt{b}{ch}")
            for j in range(4):
                k = 4 * ch + j
                nc.tensor.transpose(_fr(ps_t[:, 128 * j:128 * (j + 1)]),
                                    _fr(xpa[b][:, MPHI * k:MPHI * (k + 1)]),
                                    _fr(ident[:, :]))
            nc.vector.tensor_copy(_fr(xpT[b][:, 512 * ch:512 * (ch + 1)]), ps_t[:])

    def stile(b, t8):
        """one S^T tile + exp -> E^T"""
        ps_st = ps_big.tile([128, HW], FP, tag="ps_big", name=f"psst{b}{t8}")
        for nh in range(2):
            mm(ps_st[:, 512 * nh:512 * (nh + 1)],
               Me[b][:, 128 * t8:128 * (t8 + 1)],
               xpT[b][:, 512 * nh:512 * (nh + 1)], start=True, stop=True)
        nc.scalar.activation(_fr(ET[b][t8][:]), ps_st[:], AF.Exp)

    def exv_tile(b, pt):
        """one EXV p-tile: E @ xv_ext (col 256 = D); spiral; transpose"""
        ps_e = ps_exv.tile([128, C + 2], FP, tag="ps_exv", name=f"pse{b}{pt}")
        for k in range(8):
            mm(ps_e[:], ET[b][k][:, 128 * pt:128 * (pt + 1)],
               xvt[b][:, (C + 2) * k:(C + 2) * (k + 1)],
               start=(k == 0), stop=(k == 7))
        negD = sm.tile([128, 1], FP, tag="negD", name=f"negD{b}{pt}")
        nc.vector.tensor_scalar_mul(negD[:], ps_e[:, C:C + 1], -1.0)
        nrd = sm.tile([128, 1], FP, tag="nrd", name=f"nrd{b}{pt}")
        nc.vector.reciprocal(nrd[:], negD[:])
        spr = sprp.tile([128, C], FP, tag="spr", name=f"spr{b}{pt}")
        nc.vector.scalar_tensor_tensor(
            _fr(spr[:]), ps_e[:, 0:C], nrd[:, 0:1],
            xvt[b][:, (C + 2) * pt:(C + 2) * pt + C], op0=ALU.mult, op1=ALU.add)
        ps_tp = ps_sml.tile([128, 256], FP, tag="ps_sml", name=f"pstp{b}{pt}")
        for ch in range(2):
            nc.tensor.transpose(_fr(ps_tp[:, 128 * ch:128 * (ch + 1)]),
                                _fr(spr[:, 128 * ch:128 * (ch + 1)]),
                                _fr(ident[:, :]))
        for ch in range(2):
            nc.vector.tensor_copy(_fr(spT[b][ch][:, 128 * pt:128 * (pt + 1)]),
                                  ps_tp[:, 128 * ch:128 * (ch + 1)])

    def sa_of(b):
        """sa = softmax over pixels of w_in . x  -> [1, 1024]"""
        ps_a = ps_big.tile([1, HW], FP, tag="ps_big", name=f"psa{b}")
        wins = (W("win0"), W("win1"))
        for ki in range(2):
            for nh in range(2):
                mm(ps_a[:, 512 * nh:512 * (nh + 1)], wins[ki],
                   xmat[b][:, ki * HW + 512 * nh:ki * HW + 512 * (nh + 1)],
                   start=(ki == 0), stop=(ki == 1))
        ea = sm.tile([1, HW], FP, tag="ea", name=f"ea{b}", bufs=1)
        sae = sm.tile([1, 1], FP, tag=f"sae{b}", name=f"sae{b}", bufs=1)
        nc.scalar.activation(ea[:], ps_a[:], AF.Exp, accum_out=sae[:, 0:1])
        sar = sm.tile([1, 1], FP, tag=f"sar{b}", name=f"sar{b}", bufs=1)
        nc.vector.reciprocal(sar[:], sae[:])
        sa = sm.tile([1, HW], FP, tag="sa", name=f"sa{b}", bufs=1)
        nc.vector.tensor_scalar_mul(_fr(sa[:]), ea[:], sar[:, 0:1])
        return sa

    def out_nh(b, ot, nh, sa, WlgT, fa):
        """one 512-pixel half of an output channel block: 4 accumulated
        matmuls (conv, rank-1 app term, +x residual) -> relu -> store"""
        sl = slice(512 * nh, 512 * (nh + 1))
        ps_o = ps_big.tile([128, 512], FP, tag="ps_big", name=f"pso{b}{ot}{nh}")
        for ct in range(2):
            mm(ps_o[:], WlgT[ct][:, 128 * ot:128 * (ot + 1)],
               spT[b][ct][:, sl], start=(ct == 0), stop=False)
        mm(ps_o[:], fa[0:1, 128 * ot:128 * (ot + 1)], sa[0:1, sl],
           start=False, stop=False)
        mm(ps_o[:], ident[:, :],
           xmat[b][:, ot * HW + 512 * nh:ot * HW + 512 * (nh + 1)],
           start=False, stop=True)
        ob = obp.tile([128, 512], FP, tag="ob", name=f"ob{b}{ot}{nh}")
        if nh == 0:
            nc.scalar.activation(ob[:], ps_o[:], AF.Relu)
        else:
            nc.vector.tensor_scalar_max(ob[:], ps_o[:], 0.0)
        nc.sync.dma_start(out_cq[b, 128 * ot:128 * (ot + 1), sl], ob[:])

    # ---------------- semantic branch (batch independent) ----------------

    def sem_attention(t0=0.014, dt=0.0012):
        step = [t0]

        def nxt():
            step[0] += dt
            return step[0]

        # qT, kT [300, 20] chunks: qT = wq^T @ emb^T (+ bias col)
        embTe = [W("embTe0"), W("embTe1"), W("embTe2")]

        def qt_like(wname, bname, tag):
            outs = []
            for mi, (mo, ms) in enumerate(_ksl(DE)):
                ps = ps_sml.tile([ms, NN], FP, tag="ps_sml", name=f"ps{tag}{mi}")
                for ki, (ko, ks) in enumerate(_ksl(DE)):
                    mm(ps[:], W(f"{wname}{ki}")[:, mo:mo + ms], embTe[ki][0:ks, :],
                       start=(ki == 0), stop=(ki == 2))
                t = sm.tile([ms, NN], FP, tag=f"{tag}{mi}", name=f"{tag}{mi}", bufs=1)
                nc.scalar.activation(_fr(t[:]), ps[:], AF.Identity,
                                     bias=W(f"{bname}{mi}"))
                outs.append(t)
            return outs

        with tc.tile_wait_until(step[0]):
            qT = qt_like("wq", "bq", "qT")
        with tc.tile_wait_until(nxt()):
            kT = qt_like("wk", "bk", "kT")

        # v natural [20, 300] = embTe.T @ wve (bias row fused)
        ctx2 = tc.tile_wait_until(nxt()); ctx2.__enter__()
        ps = ps_sml.tile([NN, DE], FP, tag="ps_sml", name="psv")
        for ki in range(3):
            mm(ps[:], embTe[ki], W(f"wve{ki}"), start=(ki == 0), stop=(ki == 2))
        v_sb = sm.tile([NN, DE], FP, tag="v_sb", name="v_sb", bufs=1)
        nc.vector.tensor_copy(_fr(v_sb[:]), ps[:])

        ctx2.__exit__(None, None, None)
        ctx2 = tc.tile_wait_until(nxt()); ctx2.__enter__()
        # att = softmax(q @ k.T / sqrt(300)): logits are tiny, no max shift
        ps = ps_sml.tile([NN, NN], FP, tag="ps_sml", name="psatt")
        for ki in range(3):
            mm(ps[:], qT[ki][:, :], kT[ki][:, :], start=(ki == 0), stop=(ki == 2))
        att_e = sm.tile([NN, NN], FP, tag="att_e", name="att_e", bufs=1)
        rs_ = sm.tile([NN, 1], FP, tag="rs_", name="rs_", bufs=1)
        nc.scalar.activation(att_e[:], ps[:], AF.Exp,
                             scale=float(1.0 / np.sqrt(DE)), accum_out=rs_[:, 0:1])
        rr = sm.tile([NN, 1], FP, tag="rr", name="rr", bufs=1)
        nc.vector.reciprocal(rr[:], rs_[:])
        att_n = sm.tile([NN, NN], FP, tag="att_n", name="att_n", bufs=1)
        nc.vector.tensor_scalar_mul(att_n[:], att_e[:], rr[:, 0:1])

        ctx2.__exit__(None, None, None)
        ctx2 = tc.tile_wait_until(nxt()); ctx2.__enter__()
        # attT; AV = att @ v; node1col = AV^T @ (1/20)
        ps = ps_sml.tile([NN, NN], FP, tag="ps_sml", name="psattT")
        nc.tensor.transpose(ps[:], att_n[:], identr[0:NN, 0:NN])
        attT = sm.tile([NN, NN], FP, tag="attT", name="attT", bufs=1)
        nc.vector.tensor_copy(_fr(attT[:]), ps[:])
        ps = ps_sml.tile([NN, DE], FP, tag="ps_sml", name="psav")
        mm(ps[:], attT[:, :], v_sb[:, :], start=True, stop=True)
        av_sb = sm.tile([NN, DE], FP, tag="av_sb", name="av_sb", bufs=1)
        nc.vector.tensor_copy(_fr(av_sb[:]), ps[:])

        ctx2.__exit__(None, None, None)
        ctx2 = tc.tile_wait_until(nxt()); ctx2.__enter__()
        n1c = sm.tile([128, 3], FP, tag="n1c", name="n1c", bufs=1)
        for mi, (mo, ms) in enumerate(_ksl(DE)):
            ps = ps_sml.tile([ms, 8], FP, tag="ps_sml", name=f"psn1{mi}")
            mm(ps[:], av_sb[:, mo:mo + ms], inv20[:, :], start=True, stop=True)
            nc.scalar.copy(_fr(n1c[0:ms, mi:mi + 1]), ps[:, 0:1])

        ctx2.__exit__(None, None, None)
        ctx2 = tc.tile_wait_until(nxt()); ctx2.__enter__()
        # node2 [1,300] = node1^T @ wo + bo; ev = emb + bcast(node2)
        ps = ps_sml.tile([1, DE], FP, tag="ps_sml", name="psn2")
        for ki, (ko, ks) in enumerate(_ksl(DE)):
            mm(ps[:], n1c[0:ks, ki:ki + 1], W(f"wo{ki}"), start=(ki == 0), stop=(ki == 2))
        n2 = sm.tile([1, DE], FP, tag="n2", name="n2", bufs=1)
        nc.vector.tensor_add(_fr(n2[:]), W("bo_row"), ps[:])
        ps = ps_sml.tile([NN, DE], FP, tag="ps_sml", name="psev")
        mm(ps[:], one_row20[:, :], n2[:, :], start=True, stop=True)
        ev_sb = sm.tile([NN, DE], FP, tag="ev_sb", name="ev_sb", bufs=1)
        nc.vector.tensor_add(ev_sb[:], W("emb"), ps[:])

        ctx2.__exit__(None, None, None)
        ctx2 = tc.tile_wait_until(nxt()); ctx2.__enter__()
        # evT chunks [<=128, 20]
        evT = []
        for mi, (mo, ms) in enumerate(_ksl(DE)):
            ps = ps_sml.tile([ms, NN], FP, tag="ps_sml", name=f"psevT{mi}")
            nc.tensor.transpose(ps[:], ev_sb[:, mo:mo + ms], identr[0:NN, 0:NN])
            t = sm.tile([ms, NN], FP, tag=f"evT{mi}", name=f"evT{mi}", bufs=1)
            nc.vector.tensor_copy(_fr(t[:]), ps[:])
            evT.append(t)
        ctx2.__exit__(None, None, None)
        return evT, step[0]

    def sem_adjn():
        # adj_n = (d (x) d) * (adj + I); rsqrt via Newton on DVE (no Sqrt table)
        ah = sm.tile([NN, NN], FP, tag="ah", name="ah", bufs=1)
        nc.vector.tensor_add(ah[:], W("adj"), ident[0:NN, 0:NN])
        r20 = sm.tile([NN, 1], FP, tag="r20", name="r20", bufs=1)
        nc.vector.tensor_reduce(r20[:], ah[:], axis=mybir.AxisListType.X, op=ALU.add)
        u = sm.tile([NN, 1], FP, tag="u20", name="u20", bufs=1)
        nc.vector.reciprocal(u[:], r20[:])       # u = 1/r in [1/17, 1/5]
        u2 = sm.tile([NN, 1], FP, tag="u2_20", name="u2_20", bufs=1)
        nc.vector.tensor_scalar_mul(u2[:], u[:], 0.5)
        y = sm.tile([NN, 1], FP, tag="y20", name="y20", bufs=1)
        nc.vector.tensor_scalar(y[:], u[:], 1.23, 0.2, op0=ALU.mult, op1=ALU.add)
        for it in range(3):                      # y <- 0.5*y + (0.5*u)/y
            ry = sm.tile([NN, 1], FP, tag=f"ry{it}", name=f"ry{it}", bufs=1)
            nc.vector.reciprocal(ry[:], y[:])
            t_ = sm.tile([NN, 1], FP, tag=f"t20_{it}", name=f"t20_{it}", bufs=1)
            nc.vector.tensor_mul(t_[:], u2[:], ry[:])
            y_n = sm.tile([NN, 1], FP, tag=f"y20_{it}", name=f"y20_{it}", bufs=1)
            nc.vector.scalar_tensor_tensor(y_n[:], y[:], 0.5, t_[:],
                                           op0=ALU.mult, op1=ALU.add)
            y = y_n
        ps = ps_sml.tile([1, NN], FP, tag="ps_sml", name="psdT")
        nc.tensor.transpose(ps[:], y[:, 0:1], identr[0:NN, 0:NN])
        dT = sm.tile([1, NN], FP, tag="dT", name="dT", bufs=1)
        nc.vector.tensor_copy(_fr(dT[:]), ps[:])
        ps = ps_sml.tile([NN, NN], FP, tag="ps_sml", name="psdd")
        mm(ps[:], dT[:, :], dT[:, :], start=True, stop=True)
        adjn = sm.tile([NN, NN], FP, tag="adjn", name="adjn", bufs=1)
        nc.vector.tensor_mul(adjn[:], ah[:], ps[:])
        ps = ps_sml.tile([NN, NN], FP, tag="ps_sml", name="psadjnT")
        nc.tensor.transpose(ps[:], adjn[:], identr[0:NN, 0:NN])
        adjnT = sm.tile([NN, NN], FP, tag="adjnT", name="adjnT", bufs=1)
        nc.vector.tensor_copy(_fr(adjnT[:]), ps[:])
        return adjnT

    def sem_gcn(evT, adjnT, t0=0.024, dt=0.0012):
        step = [t0]

        def nxt():
            step[0] += dt
            return step[0]

        # GCN layer 1: g1 = relu(adj_n @ (ev @ gc1_w))
        ctx2 = tc.tile_wait_until(step[0]); ctx2.__enter__()
        ps = ps_sml.tile([NN, C], FP, tag="ps_sml", name="pst1")
        for ki in range(3):
            mm(ps[:], evT[ki][:, :], W(f"gc1{ki}"), start=(ki == 0), stop=(ki == 2))
        t1 = sm.tile([NN, C], FP, tag="t1", name="t1", bufs=1)
        nc.vector.tensor_copy(_fr(t1[:]), ps[:])
        ps = ps_sml.tile([NN, C], FP, tag="ps_sml", name="psg1")
        mm(ps[:], adjnT[:, :], t1[:, :], start=True, stop=True)
        g1 = sm.tile([NN, C], FP, tag="g1", name="g1", bufs=1)
        nc.scalar.activation(g1[:], ps[:], AF.Relu)

        ctx2.__exit__(None, None, None)
        ctx2 = tc.tile_wait_until(nxt()); ctx2.__enter__()
        g1T = []
        for mi, (mo, ms) in enumerate(_ksl(C)):
            ps = ps_sml.tile([ms, NN], FP, tag="ps_sml", name=f"psg1T{mi}")
            nc.tensor.transpose(ps[:], g1[:, mo:mo + ms], identr[0:NN, 0:NN])
            t = sm.tile([ms, NN], FP, tag=f"g1T{mi}", name=f"g1T{mi}", bufs=1)
            nc.vector.tensor_copy(_fr(t[:]), ps[:])
            g1T.append(t)

        ctx2.__exit__(None, None, None)
        ctx2 = tc.tile_wait_until(nxt()); ctx2.__enter__()
        ps = ps_sml.tile([NN, C], FP, tag="ps_sml", name="pst2")
        for ki in range(2):
            mm(ps[:], g1T[ki][:, :], W(f"gc2{ki}"), start=(ki == 0), stop=(ki == 1))
        t2 = sm.tile([NN, C], FP, tag="t2", name="t2", bufs=1)
        nc.vector.tensor_copy(_fr(t2[:]), ps[:])
        ps = ps_sml.tile([NN, C], FP, tag="ps_sml", name="psg2")
        mm(ps[:], adjnT[:, :], t2[:, :], start=True, stop=True)
        g2 = sm.tile([NN, C], FP, tag="g2", name="g2", bufs=1)
        nc.scalar.activation(_fr(g2[:]), ps[:], AF.Relu)

        ctx2.__exit__(None, None, None)
        ctx2 = tc.tile_wait_until(nxt()); ctx2.__enter__()
        # reluG [128, 2]: column cb = relu(sum_m g2[m, 128cb:...])
        reluG = sm.tile([128, 2], FP, tag="reluG", name="reluG", bufs=1)
        for cb in range(2):
            ps = ps_sml.tile([128, 8], FP, tag="ps_sml", name=f"psrg{cb}")
            mm(ps[:], g2[:, 128 * cb:128 * (cb + 1)], ones20[:, :],
               start=True, stop=True)
            nc.scalar.activation(_fr(reluG[:, cb:cb + 1]), ps[:, 0:1], AF.Relu)

        # fa [1, 256] = reluG^T @ Wa^T   (Wa^T = final_wT rows 256:512)
        ps = ps_sml.tile([1, C], FP, tag="ps_sml", name="psfa")
        for cb in range(2):
            mm(ps[:], reluG[:, cb:cb + 1], W(f"fwT{2 + cb}"),
               start=(cb == 0), stop=(cb == 1))
        fa = sm.tile([1, C], FP, tag="fa", name="fa", bufs=1)
        nc.vector.tensor_copy(_fr(fa[:]), ps[:])

        # WlgT [256, 256]: WlgT[c,o] = sum_k gw_w[k,c] Wl^T[k,o]
        WlgT = []
        for cb in range(2):
            ps = ps_sml.tile([128, C], FP, tag="ps_sml", name=f"psWlg{cb}")
            for ki in range(2):
                mm(ps[:], W(f"gww{ki}")[:, 128 * cb:128 * (cb + 1)], W(f"fwT{ki}"),
                   start=(ki == 0), stop=(ki == 1))
            t = sm.tile([128, C], FP, tag=f"WlgT{cb}", name=f"WlgT{cb}", bufs=1)
            nc.vector.tensor_copy(_fr(t[:]), ps[:])
            WlgT.append(t)
        ctx2.__exit__(None, None, None)
        return fa, WlgT

    # ================= emission schedule =================
    # tile_wait_until pins the TileScheduler's per-engine queue order to
    # this stage sequence (it otherwise reorders by its own readiness
    # model and can head-block an engine on a later-arriving DMA)
    import json
    scale = float(os.environ.get("KSCALE", "1.0"))
    times = json.loads(os.environ.get("KSCHED", "null")) or {
        "front0": 0, "dgme0": 0, "front1": 0, "xpt0": 0, "sa0": 0.008,
        "stiles0": 0, "semA": 0.012, "adjn": 0.012, "dgme1": 0.011,
        "xpt1": 0.012, "sa1": 0, "zip1": 0.02, "gcn": 0.025, "zip2": 0,
        "out1": 0, "trickle1": 0.010, "trickle2": 0.013,
        "xv0": 0.014, "xv1": 0.017}
    times = {k: v * scale for k, v in times.items()}
    for rep in range(reps):
        with tc.tile_wait_until(times["front0"]):
            xm0 = front(0, rep * BPC + 0)
        with tc.tile_wait_until(times["dgme0"]):
            dgme(0, xm0)
        with tc.tile_wait_until(times["trickle1"]):
            trickle(1, step=1240, start=W1HEAD)
        with tc.tile_wait_until(times["front1"]):
            xm1 = front(1, rep * BPC + 1)
        with tc.tile_wait_until(times["trickle2"]):
            trickle(2, step=1440)
        with tc.tile_wait_until(times["xv0"]):
            load_xv(0)
        with tc.tile_wait_until(times["xv1"]):
            load_xv(1)
        with tc.tile_wait_until(times["sa1"]):
            sa1 = sa_of(1)
        with tc.tile_wait_until(times["xpt0"]):
            xpt(0)
        with tc.tile_wait_until(times["sa0"]):
            sa0 = sa_of(0)
        with tc.tile_wait_until(times["stiles0"]):
            for t8 in range(8):
                stile(0, t8)
        evT, _semA_end = sem_attention(t0=times["semA"], dt=times.get("semdt", 0.0012))
        with tc.tile_wait_until(times["adjn"]):
            adjnT = sem_adjn()
        with tc.tile_wait_until(times["dgme1"]):
            dgme(1, xm1)
        with tc.tile_wait_until(times["xpt1"]):
            xpt(1)
        with tc.tile_wait_until(times["xv1"]):
            load_xv(1)
        with tc.tile_wait_until(times["sa1"]):
            sa1 = sa_of(1)
        zdt = times.get("zipdt", 0.0012)
        for i in range(8):
            with tc.tile_wait_until(times["zip1"] + i * zdt):
                stile(1, i)
            with tc.tile_wait_until(times["zip1"] + i * zdt + zdt / 2):
                exv_tile(0, i)
        fa, WlgT = sem_gcn(evT, adjnT, t0=times["gcn"],
                           dt=times.get("semdt", 0.0012))
        with tc.tile_wait_until(times["zip2"]):
            for i in range(8):
                exv_tile(1, i)
                if i == 1:
                    out_nh(0, 0, 0, sa0, WlgT, fa)
                    out_nh(0, 1, 0, sa0, WlgT, fa)
                if i == 3:
                    out_nh(0, 0, 1, sa0, WlgT, fa)
                    out_nh(0, 1, 1, sa0, WlgT, fa)
                if i == 5:
                    out_nh(1, 0, 0, sa1, WlgT, fa)
                    out_nh(1, 1, 0, sa1, WlgT, fa)
        with tc.tile_wait_until(times["out1"]):
            out_nh(1, 0, 1, sa1, WlgT, fa)
            out_nh(1, 1, 1, sa1, WlgT, fa)
